# revision 36
# baseline (speedup 1.0000x reference)
"""MoE transformer layer (top-2 of 16 experts) on 8 Trainium2 NeuronCores.

Strategy (expert-parallel, per the sharding hint):
  - Host computes the (tiny) gating matmul x@w_gate [2048,16], top-2 selection,
    softmax gate weights, and the aux importance loss. This is the dispatch
    step of expert-parallel MoE: it decides which tokens go to which core.
  - Each of the 8 cores owns 2 experts. The host gathers each expert's routed
    tokens (padded to a fixed capacity C=320, actual max count is 315),
    pre-transposed to [D, C] layout, and ships them with that core's expert
    weights.
  - The device kernel does the heavy work: per expert, h = relu(w1^T x + b1)
    and y = w2^T h, as 128x128xC matmuls. Weights and activations are bf16
    (weights stream from HBM exactly once, so bf16 halves the dominant
    traffic; PSUM accumulation stays fp32; measured output rel err ~3e-3).
    The h-phase runs d-major over two 8-bank PSUM halves so the PE starts
    as soon as the first w1 tile lands; the y-phase runs d-major so PSUM
    copies and output DMAs overlap later groups. Loads are issued on the
    SP HWDGE queue, outputs on the scalar-engine HWDGE queue, so stores
    never stall the load stream. Measured: ~55.6us HW exec per core
    (PE busy ~37us saturated, load stream ~28us busy).
  - Host combine: out[tok] += g * (y + b2) for each expert's token list
    (the gate-weighted scatter-add; equivalent to the all-reduce combine).
"""

import sys

for _p in (
    "/opt/trn_rl_repo",
    "/root/.axon_site",
    "/root/.axon_site/_ro/pypackages",
    "/root/.axon_site/_ro/trn_rl_repo",
):
    if _p not in sys.path:
        sys.path.append(_p)

from contextlib import ExitStack

import numpy as np

import concourse.bass as bass
import concourse.mybir as mybir
import concourse.tile as tile
from concourse import bacc
from concourse.bass import ts
from concourse.bass_utils import run_bass_kernel_spmd

P, B, D, H, E = 64, 32, 512, 2048, 16
T = P * B                    # 2048 tokens
TOP_K = 2
IMPORTANCE_FACTOR = 0.01
EPS = 1e-10

NCORES = 8
EPG = E // NCORES            # experts per core
C = 320                      # per-expert token capacity (actual max 315)
DT, HT = D // 128, H // 128  # 4, 16

F32 = mybir.dt.float32
FR = mybir.dt.float32r
BF16 = mybir.dt.bfloat16
# Weight dtype: bfloat16 halves the dominant HBM traffic (weights are
# streamed once); activations stay float32r. Set to FR for full precision.
WDT = BF16


def build_nc():
    nc = bacc.Bacc()
    xT = nc.declare_dram_parameter("xT", [EPG, D, C], WDT, isOutput=False)
    w1 = nc.declare_dram_parameter("w1", [EPG, D, H], WDT, isOutput=False)
    b1t = nc.declare_dram_parameter("b1t", [EPG, 128, HT], F32, isOutput=False)
    w2 = nc.declare_dram_parameter("w2", [EPG, H, D], WDT, isOutput=False)
    yT = nc.declare_dram_parameter("yT", [EPG, D, C], F32, isOutput=True)

    with tile.TileContext(nc) as tc, ExitStack() as ctx:
        x_pool = ctx.enter_context(tc.tile_pool(name="x", bufs=2 * DT))
        w1_pool = ctx.enter_context(tc.tile_pool(name="w1", bufs=16))
        w2_pool = ctx.enter_context(tc.tile_pool(name="w2", bufs=8))
        h_pool = ctx.enter_context(tc.tile_pool(name="h", bufs=HT + 4))
        y_pool = ctx.enter_context(tc.tile_pool(name="y", bufs=4))
        b_pool = ctx.enter_context(tc.tile_pool(name="b", bufs=2))
        ps_pool = ctx.enter_context(tc.tile_pool(name="ps", bufs=8, space="PSUM"))

        # Each dma_start costs ~0.7us of HWDGE sequencer issue time, so w2
        # is batched 4 h-tiles per transfer. w1 stays one d-tile per DMA so
        # the PE can start after the FIRST 512KB rather than the whole 2MB.
        W2M = 4   # h-tiles per w2 DMA -> 4 DMAs/expert
        HH = HT // 2  # h-phase half size (PSUM has 8 banks)

        for e in range(EPG):
            # interleave x/w1 issue so the first accumulation group's data
            # (x0 + w1_0, 0.58MB) lands as early as possible; b1 is only
            # needed by the relu, so it is issued after the weights.
            # w1 is split column-wise into the two h-phase halves: half 0's
            # matmuls only read w1 columns 0..HH*128, so loading halves as
            # separate DMAs lets the PE start after 256KB instead of 512KB,
            # and half-1 columns stream in behind half 0's compute.
            HHC = HH * 128
            xts, w1hs = [], [[], []]
            for d in range(DT):
                xt = x_pool.tile([128, C], WDT, tag="xt")
                nc.sync.dma_start(xt[:], xT[e, ts(d, 128), :])
                xts.append(xt)
                w1a = w1_pool.tile([128, HHC], WDT, tag="w1a")
                nc.sync.dma_start(w1a[:], w1[e, ts(d, 128), 0:HHC])
                w1hs[0].append(w1a)
            for d in range(DT):
                w1b = w1_pool.tile([128, HHC], WDT, tag="w1b")
                nc.sync.dma_start(w1b[:], w1[e, ts(d, 128), HHC:H])
                w1hs[1].append(w1b)

            b1_t = b_pool.tile([128, HT], F32)
            nc.sync.dma_start(b1_t[:], b1t[e])

            # h-phase in two d-major halves of 8 PSUM banks each: the d=0
            # matmuls of a half need only w1 tile d (512KB), so PE work
            # starts as soon as the first w1 DMA lands and consumes each
            # w1 tile at the rate it arrives.
            hts = []
            for half in range(2):
                psh = [
                    ps_pool.tile([128, C], F32, tag="ps", name=f"psh{e}_{half}_{i}")
                    for i in range(HH)
                ]
                for d in range(DT):
                    for i in range(HH):
                        h = half * HH + i
                        nc.tensor.matmul(
                            psh[i][:],
                            w1hs[half][d][:, ts(i, 128)],
                            xts[d][:],
                            start=(d == 0),
                            stop=(d == DT - 1),
                            skip_group_check=True,
                        )
                for i in range(HH):
                    h = half * HH + i
                    ht_t = h_pool.tile([128, C], WDT, tag="ht")
                    nc.scalar.activation(
                        ht_t[:], psh[i][:], mybir.ActivationFunctionType.Relu,
                        bias=b1_t[:, h:h + 1],
                    )
                    hts.append(ht_t)

            w2ms = []
            for a in range(HT // W2M):
                w2m = w2_pool.tile([128, W2M * D], WDT, tag="w2m")
                nc.sync.dma_start(
                    w2m[:].rearrange("p (j d) -> p j d", j=W2M),
                    w2[e, ts(a, W2M * 128), :].rearrange("(j p) d -> p j d", p=128),
                )
                w2ms.append(w2m)

            def w2_slice(h, d):
                return w2ms[h // W2M][:, (h % W2M) * D + d * 128:(h % W2M) * D + (d + 1) * 128]

            # y-phase d-major: groups finish one at a time, so each PSUM
            # copy + output DMA overlaps the next group's matmuls.
            for d in range(DT):
                ps = ps_pool.tile([128, C], F32, tag="ps")
                for h in range(HT):
                    nc.tensor.matmul(
                        ps[:],
                        w2_slice(h, d),
                        hts[h][:],
                        start=(h == 0),
                        stop=(h == HT - 1),
                    )
                yt = y_pool.tile([128, C], F32, tag="yt")
                nc.vector.tensor_copy(yt[:], ps[:])
                nc.scalar.dma_start(yT[e, ts(d, 128), :], yt[:])
    # Bacc legalization: TRN2 allows 1 sync wait per instruction; this
    # splits the rest into event-semaphore instructions, allocates regs, etc.
    nc.compile()
    return nc


_NC_CACHE = None


def _get_nc():
    global _NC_CACHE
    if _NC_CACHE is None:
        _NC_CACHE = build_nc()
    return _NC_CACHE


def route(x, w_gate):
    """Host gating: returns (idx per expert, gate weight per expert, aux_loss)."""
    xf = np.ascontiguousarray(np.asarray(x, np.float32).reshape(T, D))
    wg = np.asarray(w_gate, np.float32)
    v = xf @ wg                                        # [T, E] float32
    order = np.argsort(-v, axis=1, kind="stable")      # ties: lowest index first
    i1, i2 = order[:, 0], order[:, 1]
    m1 = v[np.arange(T), i1]
    m2 = v[np.arange(T), i2]
    # softmax over the top-2 values (matches jax.nn.softmax on fp32)
    e2 = np.exp(m2 - m1)
    g1 = 1.0 / (1.0 + e2)
    g2 = e2 / (1.0 + e2)
    gates = np.zeros((T, E), np.float32)
    gates[np.arange(T), i1] = g1
    gates[np.arange(T), i2] = g2
    importance = gates.sum(axis=0, dtype=np.float32)
    imp_mean = importance.mean()
    imp_var = importance.var(ddof=1)
    aux = np.float32(IMPORTANCE_FACTOR * imp_var / (imp_mean**2 + EPS))
    idx = [np.where((i1 == e) | (i2 == e))[0].astype(np.int32) for e in range(E)]
    gsel = [gates[idx[e], e] for e in range(E)]
    return xf, idx, gsel, aux


def prepare(x, w_gate, w1, b1, w2, b2):
    """Host dispatch: routing + per-core input maps. Returns (in_maps, state)."""
    wnp = mybir.dt.np(WDT)
    w1 = np.ascontiguousarray(np.asarray(w1, np.float32)).astype(wnp)
    b1 = np.asarray(b1, np.float32)
    w2 = np.ascontiguousarray(np.asarray(w2, np.float32)).astype(wnp)
    b2 = np.asarray(b2, np.float32)

    xf, idx, gsel, aux = route(x, w_gate)

    # b1 pre-transposed per expert to [128, HT] so bias is a per-partition AP
    b1t = np.ascontiguousarray(
        b1.reshape(E, HT, 128).transpose(0, 2, 1)
    )  # [E, 128, HT]

    in_maps = []
    for m in range(NCORES):
        es = [EPG * m + j for j in range(EPG)]
        xT_host = np.zeros((EPG, D, C), wnp)
        for j, e in enumerate(es):
            ntok = len(idx[e])
            xT_host[j, :, :ntok] = xf[idx[e]].T.astype(wnp)
        in_maps.append({
            "xT": xT_host,
            "w1": w1[es],
            "b1t": b1t[es],
            "w2": w2[es],
        })
    return in_maps, (idx, gsel, aux, b2)


def combine(results, state):
    """Host combine: gate-weighted scatter-add of per-expert outputs."""
    idx, gsel, aux, b2 = state
    out = np.zeros((T, D), np.float32)
    for m in range(NCORES):
        yT = results[m]["yT"]  # [EPG, D, C]
        for j in range(EPG):
            e = EPG * m + j
            ntok = len(idx[e])
            y = yT[j, :, :ntok].T + b2[e]          # [ntok, D]
            out[idx[e]] += gsel[e][:, None] * y
    return out.reshape(P, B, D), aux


def kernel(x, w_gate, w1, b1, w2, b2):
    in_maps, state = prepare(x, w_gate, w1, b1, w2, b2)
    res = run_bass_kernel_spmd(_get_nc(), in_maps, list(range(NCORES)))
    return combine(res.results, state)


# revision 39
# speedup vs baseline: 1.0539x; 1.0539x over previous
"""MoE transformer layer (top-2 of 16 experts) on 8 Trainium2 NeuronCores.

Strategy (expert-parallel, per the sharding hint):
  - Host computes the (tiny) gating matmul x@w_gate [2048,16], top-2 selection,
    softmax gate weights, and the aux importance loss. This is the dispatch
    step of expert-parallel MoE: it decides which tokens go to which core.
  - Each of the 8 cores owns 2 experts. The host gathers each expert's routed
    tokens (padded to a fixed capacity C=320, actual max count is 315),
    pre-transposed to [D, C] layout, and ships them with that core's expert
    weights.
  - The device kernel does the heavy work: per expert, h = relu(w1^T x + b1)
    and y = w2^T h, as 128x128xC matmuls. Weights and activations are bf16
    (weights stream from HBM exactly once, so bf16 halves the dominant
    traffic; PSUM accumulation stays fp32; measured output rel err ~3e-3).
    The h-phase runs d-major over two 8-bank PSUM halves so the PE starts
    as soon as the first w1 tile lands; the y-phase runs d-major so PSUM
    copies and output DMAs overlap later groups. Loads are issued on the
    SP HWDGE queue, outputs on the scalar-engine HWDGE queue, so stores
    never stall the load stream. Measured: ~55.6us HW exec per core
    (PE busy ~37us saturated, load stream ~28us busy).
  - Host combine: out[tok] += g * (y + b2) for each expert's token list
    (the gate-weighted scatter-add; equivalent to the all-reduce combine).
"""

import sys

for _p in (
    "/opt/trn_rl_repo",
    "/root/.axon_site",
    "/root/.axon_site/_ro/pypackages",
    "/root/.axon_site/_ro/trn_rl_repo",
):
    if _p not in sys.path:
        sys.path.append(_p)

from contextlib import ExitStack

import numpy as np

import concourse.bass as bass
import concourse.mybir as mybir
import concourse.tile as tile
from concourse import bacc
from concourse.bass import ts
from concourse.bass_utils import run_bass_kernel_spmd

P, B, D, H, E = 64, 32, 512, 2048, 16
T = P * B                    # 2048 tokens
TOP_K = 2
IMPORTANCE_FACTOR = 0.01
EPS = 1e-10

NCORES = 8
EPG = E // NCORES            # experts per core
C = 320                      # per-expert token capacity (actual max 315)
DT, HT = D // 128, H // 128  # 4, 16

F32 = mybir.dt.float32
FR = mybir.dt.float32r
BF16 = mybir.dt.bfloat16
# Weight dtype: bfloat16 halves the dominant HBM traffic (weights are
# streamed once); activations stay float32r. Set to FR for full precision.
WDT = BF16


def build_nc():
    nc = bacc.Bacc()
    xT = nc.declare_dram_parameter("xT", [EPG, D, C], WDT, isOutput=False)
    w1 = nc.declare_dram_parameter("w1", [EPG, D, H], WDT, isOutput=False)
    b1t = nc.declare_dram_parameter("b1t", [EPG, 128, HT], F32, isOutput=False)
    w2 = nc.declare_dram_parameter("w2", [EPG, H, D], WDT, isOutput=False)
    yT = nc.declare_dram_parameter("yT", [EPG, D, C], F32, isOutput=True)

    with tile.TileContext(nc) as tc, ExitStack() as ctx:
        x_pool = ctx.enter_context(tc.tile_pool(name="x", bufs=2 * DT))
        w1_pool = ctx.enter_context(tc.tile_pool(name="w1", bufs=8))
        w2_pool = ctx.enter_context(tc.tile_pool(name="w2", bufs=8))
        h_pool = ctx.enter_context(tc.tile_pool(name="h", bufs=HT + 4))
        y_pool = ctx.enter_context(tc.tile_pool(name="y", bufs=4))
        b_pool = ctx.enter_context(tc.tile_pool(name="b", bufs=2))
        ps_pool = ctx.enter_context(tc.tile_pool(name="ps", bufs=8, space="PSUM"))

        # Each dma_start costs ~0.7us of HWDGE sequencer issue time, so w2
        # is batched 4 h-tiles per transfer. w1 stays one d-tile per DMA so
        # the PE can start after the FIRST 512KB rather than the whole 2MB.
        W2M = 4   # h-tiles per w2 DMA -> 4 DMAs/expert
        HH = HT // 2  # h-phase half size (PSUM has 8 banks)

        for e in range(EPG):
            # interleave x/w1 issue so the first accumulation group's data
            # (x0 + w1_0, 0.58MB) lands as early as possible; b1 is only
            # needed by the relu, so it is issued after the weights.
            xts, w1ts = [], []
            for d in range(DT):
                xt = x_pool.tile([128, C], WDT, tag="xt")
                nc.sync.dma_start(xt[:], xT[e, ts(d, 128), :])
                xts.append(xt)
                w1t = w1_pool.tile([128, H], WDT, tag="w1t")
                nc.sync.dma_start(w1t[:], w1[e, ts(d, 128), :])
                w1ts.append(w1t)

            b1_t = b_pool.tile([128, HT], F32)
            nc.sync.dma_start(b1_t[:], b1t[e])

            # h-phase in two d-major halves of 8 PSUM banks each: the d=0
            # matmuls of a half need only w1 tile d (512KB), so PE work
            # starts as soon as the first w1 DMA lands and consumes each
            # w1 tile at the rate it arrives.
            hts = []
            for half in range(2):
                psh = [
                    ps_pool.tile([128, C], F32, tag="ps", name=f"psh{e}_{half}_{i}")
                    for i in range(HH)
                ]
                for d in range(DT):
                    for i in range(HH):
                        h = half * HH + i
                        nc.tensor.matmul(
                            psh[i][:],
                            w1ts[d][:, ts(h, 128)],
                            xts[d][:],
                            start=(d == 0),
                            stop=(d == DT - 1),
                            skip_group_check=True,
                        )
                for i in range(HH):
                    h = half * HH + i
                    ht_t = h_pool.tile([128, C], WDT, tag="ht")
                    nc.scalar.activation(
                        ht_t[:], psh[i][:], mybir.ActivationFunctionType.Relu,
                        bias=b1_t[:, h:h + 1],
                    )
                    hts.append(ht_t)

            w2ms = []
            for a in range(HT // W2M):
                w2m = w2_pool.tile([128, W2M * D], WDT, tag="w2m")
                nc.sync.dma_start(
                    w2m[:].rearrange("p (j d) -> p j d", j=W2M),
                    w2[e, ts(a, W2M * 128), :].rearrange("(j p) d -> p j d", p=128),
                )
                w2ms.append(w2m)

            def w2_slice(h, d):
                return w2ms[h // W2M][:, (h % W2M) * D + d * 128:(h % W2M) * D + (d + 1) * 128]

            # y-phase d-major: groups finish one at a time, so each PSUM
            # copy + output DMA overlaps the next group's matmuls.
            for d in range(DT):
                ps = ps_pool.tile([128, C], F32, tag="ps")
                for h in range(HT):
                    nc.tensor.matmul(
                        ps[:],
                        w2_slice(h, d),
                        hts[h][:],
                        start=(h == 0),
                        stop=(h == HT - 1),
                    )
                yt = y_pool.tile([128, C], F32, tag="yt")
                nc.vector.tensor_copy(yt[:], ps[:])
                nc.scalar.dma_start(yT[e, ts(d, 128), :], yt[:])
    # Bacc legalization: TRN2 allows 1 sync wait per instruction; this
    # splits the rest into event-semaphore instructions, allocates regs, etc.
    nc.compile()
    return nc


_NC_CACHE = None


def _get_nc():
    global _NC_CACHE
    if _NC_CACHE is None:
        _NC_CACHE = build_nc()
    return _NC_CACHE


def route(x, w_gate):
    """Host gating: returns (idx per expert, gate weight per expert, aux_loss)."""
    xf = np.ascontiguousarray(np.asarray(x, np.float32).reshape(T, D))
    wg = np.asarray(w_gate, np.float32)
    v = xf @ wg                                        # [T, E] float32
    order = np.argsort(-v, axis=1, kind="stable")      # ties: lowest index first
    i1, i2 = order[:, 0], order[:, 1]
    m1 = v[np.arange(T), i1]
    m2 = v[np.arange(T), i2]
    # softmax over the top-2 values (matches jax.nn.softmax on fp32)
    e2 = np.exp(m2 - m1)
    g1 = 1.0 / (1.0 + e2)
    g2 = e2 / (1.0 + e2)
    gates = np.zeros((T, E), np.float32)
    gates[np.arange(T), i1] = g1
    gates[np.arange(T), i2] = g2
    importance = gates.sum(axis=0, dtype=np.float32)
    imp_mean = importance.mean()
    imp_var = importance.var(ddof=1)
    aux = np.float32(IMPORTANCE_FACTOR * imp_var / (imp_mean**2 + EPS))
    idx = [np.where((i1 == e) | (i2 == e))[0].astype(np.int32) for e in range(E)]
    gsel = [gates[idx[e], e] for e in range(E)]
    return xf, idx, gsel, aux


def prepare(x, w_gate, w1, b1, w2, b2):
    """Host dispatch: routing + per-core input maps. Returns (in_maps, state)."""
    wnp = mybir.dt.np(WDT)
    w1 = np.ascontiguousarray(np.asarray(w1, np.float32)).astype(wnp)
    b1 = np.asarray(b1, np.float32)
    w2 = np.ascontiguousarray(np.asarray(w2, np.float32)).astype(wnp)
    b2 = np.asarray(b2, np.float32)

    xf, idx, gsel, aux = route(x, w_gate)

    # b1 pre-transposed per expert to [128, HT] so bias is a per-partition AP
    b1t = np.ascontiguousarray(
        b1.reshape(E, HT, 128).transpose(0, 2, 1)
    )  # [E, 128, HT]

    in_maps = []
    for m in range(NCORES):
        es = [EPG * m + j for j in range(EPG)]
        xT_host = np.zeros((EPG, D, C), wnp)
        for j, e in enumerate(es):
            ntok = len(idx[e])
            xT_host[j, :, :ntok] = xf[idx[e]].T.astype(wnp)
        in_maps.append({
            "xT": xT_host,
            "w1": w1[es],
            "b1t": b1t[es],
            "w2": w2[es],
        })
    return in_maps, (idx, gsel, aux, b2)


def combine(results, state):
    """Host combine: gate-weighted scatter-add of per-expert outputs."""
    idx, gsel, aux, b2 = state
    out = np.zeros((T, D), np.float32)
    for m in range(NCORES):
        yT = results[m]["yT"]  # [EPG, D, C]
        for j in range(EPG):
            e = EPG * m + j
            ntok = len(idx[e])
            y = yT[j, :, :ntok].T + b2[e]          # [ntok, D]
            out[idx[e]] += gsel[e][:, None] * y
    return out.reshape(P, B, D), aux


def kernel(x, w_gate, w1, b1, w2, b2):
    in_maps, state = prepare(x, w_gate, w1, b1, w2, b2)
    res = run_bass_kernel_spmd(_get_nc(), in_maps, list(range(NCORES)))
    return combine(res.results, state)
